# revision 7
# baseline (speedup 1.0000x reference)
"""Channel-attention (CAM) Trainium2 kernel.

Reference computation (per batch b of 16):
    q   = x[b].reshape(C, HW)                  # C=512, HW=4096
    sim = q @ q.T                              # [C, C], symmetric
    attn = softmax(max(sim) - sim, axis=-1)    # == exp(min_r - sim) / Z_r
    out[b] = gamma * attn @ q + x[b]

Sharding: data-parallel over batch across 8 NeuronCores (2 batches/core).
kernel() takes full inputs, shards internally, returns the full output.

Per-core kernel design (v4, fp16 matmul path):
  - All matmuls in float16: fp16 streams the PE at 1 col/cycle @2.4GHz
    (vs f32r ~1.28), FWL halves LDWEIGHTS, casts/copybacks run at 2x DVE
    rate. PSUM accumulation stays fp32. Measured end-to-end rel_l2 ~1e-3
    (gate 2e-2); bf16 would flip softmax winners (sim err ~0.3).
  - ALL transposes are REGULAR matmuls (data stationary, identity moving)
    instead of transpose-mode: same cost, but they count as PE-busy for
    the HAM clock gate, avoiding 1.2GHz re-throttle oscillation.
  - the 128x128 identity is loaded from DRAM (extra const input synthesized
    in make_in_maps) instead of an iota/affine chain, so warmup matmuls
    start ~3us earlier.
  - x is streamed in column-waves, cast f32->fp16 (DVE, 512-col slices,
    slice-major across channel blocks to cut transpose wait), PE-transposed
    into qT tiles [n, c]; sim matmuls run two chunks behind the transposes.
  - sim is symmetric: compute the exact upper-tri block rows (cols >=
    (0,128,256,384)); the 6 missing lower blocks are cast-to-fp16 +
    matmul-transposed out of the mirror blocks, interleaved per-mi with
    the softmax reduce/exp chain.
  - softmax via ACT: p = exp(min_r - sim) with accum_out producing Z in
    the same pass (fp16 out); rows scaled by gamma/Z (DVE), PE-transposed,
    identity added so mm2 computes gamma*attn@q + q = out directly.
  - batch-1's first three column waves (loads + casts) are prefetched
    before batch-0's softmax: no DVE head-of-line block, and batch-1
    transposes (real matmuls) fill the PE during batch-0's softmax.
  - mm2(0): 2-bank PSUM rotation with copybacks split in half across
    DVE+ACT so the bank frees inside the 3-matmul window (no WAR bubble).
    mm2(1): PSUM from the then-idle psim pool (4-deep, no WAR pressure),
    stores per-1024 alternating between the Sync and ACT HWDGE rings to
    halve issue serialization and shorten the tail.
  - 6 real warmup matmuls pre-warm the HAM clock gate during preamble.
"""
import sys

if "/opt/trn_rl_repo" not in sys.path:
    sys.path.insert(0, "/opt/trn_rl_repo")

import numpy as np

B, C, H, W = 16, 512, 64, 64
HW = H * W
NCORES = 8
NB = B // NCORES          # batches per core
P = 128
CB = C // P               # 4 channel blocks
KN = HW // P              # 32 contraction chunks for sim
NJ = HW // 512            # 8 output column chunks

_BUILD_CACHE = {}


def build_bass():
    import concourse.bacc as bacc
    import concourse.tile as tile
    from concourse import mybir

    f32 = mybir.dt.float32
    f16 = mybir.dt.float16
    AX = mybir.AxisListType
    ALU = mybir.AluOpType
    ACTF = mybir.ActivationFunctionType

    nc = bacc.Bacc()
    x_ext = nc.declare_dram_parameter("x", [NB, C, HW], f32, isOutput=False)
    g_ext = nc.declare_dram_parameter("gamma", [1], f32, isOutput=False)
    i_ext = nc.declare_dram_parameter("ident", [P, P], f16, isOutput=False)
    o_ext = nc.declare_dram_parameter("out", [NB, C, HW], f32, isOutput=True)

    # alternate PSUM->SBUF copies between ACT and DVE to balance engines
    _flip = [0]

    with tile.TileContext(nc) as tc:
        with (
            tc.tile_pool(name="const", bufs=1) as const,
            tc.tile_pool(name="xchunk", bufs=6) as xchunk,
            tc.tile_pool(name="qr", bufs=8) as qrp,
            tc.tile_pool(name="qt", bufs=10) as qtp,
            tc.tile_pool(name="pp", bufs=4) as pp,
            tc.tile_pool(name="osb", bufs=4) as osb,
            tc.tile_pool(name="tri", bufs=2) as trip,
            tc.tile_pool(name="vec", bufs=6) as vec,
            tc.tile_pool(name="psA", bufs=2, space="PSUM") as psA,
            tc.tile_pool(name="psim", bufs=4, space="PSUM") as psimp,
            tc.tile_pool(name="pfeat", bufs=2, space="PSUM") as pfeat,
        ):
            def copyback(dst, src):
                if _flip[0] % 2 == 0:
                    nc.scalar.copy(dst, src)
                else:
                    nc.vector.tensor_copy(dst, src)
                _flip[0] += 1

            # identity first: it unblocks the HAM warmup matmuls
            ident_h = const.tile([P, P], f16)
            nc.sync.dma_start(out=ident_h[:], in_=i_ext[:])

            # batch-0 first-wave loads go next so DMA starts during preamble
            pre_x = {}
            for mi in range(CB):
                xt = xchunk.tile([P, 1024], f32, tag="xc", name=f"prex{mi}")
                nc.sync.dma_start(
                    out=xt[:, :512], in_=x_ext[0, mi * P:(mi + 1) * P, 0:512]
                )
                pre_x[mi] = xt

            gamma_sb = const.tile([P, 1], f32)
            nc.sync.dma_start(out=gamma_sb[:], in_=g_ext[:].to_broadcast([P, 1]))

            # mm_transpose: out[P,128](f32 PSUM) = in_[P,128](f16).T via a
            # REGULAR matmul with identity moving. Counts as HAM PE-busy
            # (transpose-mode does not), same cost.
            def mm_transpose(out, in_):
                nc.tensor.matmul(out, in_, ident_h[:], start=True, stop=True)

            # real warmup matmuls while the first loads land (HAM warm-up)
            warm = psA.tile([P, C], f32, tag="psA", name="warmup")
            for i in range(6):
                nc.tensor.matmul(warm[:, :P], ident_h[:], ident_h[:],
                                 start=True, stop=True)

            # column waves per batch; first two finer to cut startup latency
            WAVES = [(0, 512), (512, 512), (1024, 1024), (2048, 1024), (3072, 1024)]
            C0S = [mi * P for mi in range(CB)]  # 0,128,256,384 (exact upper tri)
            # tri fills grouped by destination block-row
            TRI = {1: [(1, 0)], 2: [(2, 0), (2, 1)], 3: [(3, 0), (3, 1), (3, 2)]}

            def alloc_state(b):
                st = {}
                st["qr"] = [qrp.tile([P, HW], f16, tag="qr", name=f"qr{b}_{i}")
                            for i in range(CB)]
                st["done_waves"] = set()
                return st

            def load_wave(b, st, w0, wlen, use_pre=False):
                """DMA one wave, then cast in 512-col slices, slice-major
                across the 4 channel blocks (earliest transpose feed)."""
                xts = []
                for mi in range(CB):
                    if use_pre:
                        xt = pre_x[mi]
                    else:
                        xt = xchunk.tile([P, 1024], f32, tag="xc")
                        nc.sync.dma_start(
                            out=xt[:, :wlen],
                            in_=x_ext[b, mi * P:(mi + 1) * P, w0:w0 + wlen],
                        )
                    xts.append(xt)
                for s0 in range(0, wlen, 512):
                    for mi in range(CB):
                        nc.vector.tensor_copy(
                            st["qr"][mi][:, w0 + s0:w0 + s0 + 512],
                            xts[mi][:, s0:s0 + 512],
                        )
                st["done_waves"].add(w0)

            def phase1(b, st):
                """transpose to qT, sim matmuls (upper-tri)."""
                st["psim"] = [psimp.tile([P, C], f32, tag="psim",
                                         name=f"psim{b}_{i}") for i in range(CB)]
                qr_t, psim = st["qr"], st["psim"]
                qt_tiles = {}

                def mm1(kn):
                    for mi in range(CB):
                        c0 = C0S[mi]
                        nc.tensor.matmul(
                            psim[mi][:, c0:],
                            qt_tiles[kn][:, mi * P:(mi + 1) * P],
                            qt_tiles[kn][:, c0:],
                            start=(kn == 0),
                            stop=(kn == KN - 1),
                        )

                pending = []
                for (w0, wlen) in WAVES:
                    if w0 not in st["done_waves"]:
                        load_wave(b, st, w0, wlen,
                                  use_pre=(b == 0 and w0 == 0))
                    for kq in range(wlen // P):
                        kn = w0 // P + kq
                        pst = psA.tile([P, C], f32, tag="psA")
                        for ci in range(CB):
                            mm_transpose(
                                pst[:, ci * P:(ci + 1) * P],
                                qr_t[ci][:, kn * P:(kn + 1) * P],
                            )
                        qt = qtp.tile([P, C], f16, tag="qt", name=f"qt{b}_{kn}")
                        qt_tiles[kn] = qt
                        copyback(qt[:], pst[:])
                        pending.append(kn)
                        if len(pending) > 2:
                            mm1(pending.pop(0))
                for kn in pending:
                    mm1(kn)
                return st

            def softmax_pt(b, st):
                """tri fills + rowwise softmax (pipelined per block-row),
                then build lhsT = T(p*gamma/Z)+I."""
                psim = st["psim"]
                ps_t = []
                for mi in range(CB):
                    for (i, j) in TRI.get(mi, []):
                        tmp = trip.tile([P, P], f16, tag="tri")
                        nc.scalar.copy(tmp[:], psim[j][:, i * P:(i + 1) * P])
                        mm_transpose(psim[i][:, j * P:(j + 1) * P], tmp[:])
                    mrow = vec.tile([P, 1], f32, tag="mrow")
                    nc.vector.tensor_reduce(
                        mrow[:], psim[mi][:], axis=AX.X, op=ALU.min
                    )
                    zrow = vec.tile([P, 1], f32, tag="zrow")
                    p_t = pp.tile([P, C], f16, tag="p", bufs=2)
                    nc.scalar.activation(
                        p_t[:], psim[mi][:], ACTF.Exp,
                        bias=mrow[:], scale=-1.0, accum_out=zrow[:],
                    )
                    rz = vec.tile([P, 1], f32, tag="rz")
                    nc.vector.reciprocal(rz[:], zrow[:])
                    rzg = vec.tile([P, 1], f32, tag="rzg")
                    nc.vector.tensor_mul(rzg[:], rz[:], gamma_sb[:])
                    p_s = pp.tile([P, C], f16, tag="psc", bufs=4)
                    nc.vector.tensor_scalar_mul(p_s[:], p_t[:], rzg[:])
                    ps_t.append(p_s)
                pt_t = []
                for kd in range(CB):
                    pst = pfeat.tile([P, C], f32, tag="pf")
                    for ci in range(CB):
                        mm_transpose(
                            pst[:, ci * P:(ci + 1) * P],
                            ps_t[ci][:, kd * P:(kd + 1) * P],
                        )
                    t = pp.tile([P, C], f16, tag="pt")
                    copyback(t[:], pst[:])
                    nc.vector.tensor_add(
                        t[:, kd * P:(kd + 1) * P],
                        t[:, kd * P:(kd + 1) * P],
                        ident_h[:],
                    )
                    pt_t.append(t)
                st["pt"] = pt_t

            def mm2(b, st):
                """out = (gamma*diag(1/Z)*P + I) @ q, staged stores."""
                qr_t, pt_t = st["qr"], st["pt"]
                last = (b == NB - 1)
                if last:
                    # psim pool is idle now: 4-deep PSUM rotation, and
                    # per-1024 stores on alternating HWDGE rings (sync/ACT)
                    sq = [0]
                    for mi in range(CB):
                        for njp in range(NJ // 2):
                            stg = osb.tile([P, 1024], f32, tag="otf")
                            for half in range(2):
                                nj = njp * 2 + half
                                pf = psimp.tile([P, 512], f32, tag="psim")
                                for kd in range(CB):
                                    nc.tensor.matmul(
                                        pf[:],
                                        pt_t[kd][:, mi * P:(mi + 1) * P],
                                        qr_t[kd][:, nj * 512:(nj + 1) * 512],
                                        start=(kd == 0),
                                        stop=(kd == CB - 1),
                                    )
                                copyback(stg[:, half * 512:(half + 1) * 512],
                                         pf[:])
                            eng = nc.sync if sq[0] % 2 == 0 else nc.scalar
                            sq[0] += 1
                            eng.dma_start(
                                out=o_ext[b, mi * P:(mi + 1) * P,
                                          njp * 1024:(njp + 1) * 1024],
                                in_=stg[:],
                            )
                else:
                    for mi in range(CB):
                        for half in range(2):
                            stg = osb.tile([P, HW // 2], f32, tag="ot")
                            for njh in range(NJ // 2):
                                nj = half * (NJ // 2) + njh
                                pf = pfeat.tile([P, 512], f32, tag="pf")
                                for kd in range(CB):
                                    nc.tensor.matmul(
                                        pf[:],
                                        pt_t[kd][:, mi * P:(mi + 1) * P],
                                        qr_t[kd][:, nj * 512:(nj + 1) * 512],
                                        start=(kd == 0),
                                        stop=(kd == CB - 1),
                                    )
                                # split copyback across DVE+ACT so the PSUM
                                # bank frees inside the 3-matmul window
                                dst = stg[:, njh * 512:(njh + 1) * 512]
                                nc.scalar.copy(dst[:, :256], pf[:, :256])
                                nc.vector.tensor_copy(dst[:, 256:], pf[:, 256:])
                            nc.sync.dma_start(
                                out=o_ext[b, mi * P:(mi + 1) * P,
                                          half * (HW // 2):(half + 1) * (HW // 2)],
                                in_=stg[:],
                            )

            # phase-reordered emission: batch-1's first waves (loads+casts)
            # are prefetched before batch-0's softmax so the DVE queue has
            # no head-of-line block and batch-1 transposes (real matmuls)
            # fill the PE during batch-0's softmax chain. mm2(0) is emitted
            # after phase1(1) as the lower-priority PE filler.
            st0 = alloc_state(0)
            phase1(0, st0)
            st1 = alloc_state(1)
            load_wave(1, st1, 0, 512)
            load_wave(1, st1, 512, 512)
            load_wave(1, st1, 1024, 1024)
            softmax_pt(0, st0)
            phase1(1, st1)
            mm2(0, st0)
            softmax_pt(1, st1)
            mm2(1, st1)

    nc.finalize()
    return nc


def get_bass():
    if "nc" not in _BUILD_CACHE:
        _BUILD_CACHE["nc"] = build_bass()
    return _BUILD_CACHE["nc"]


_IDENT = None


def make_in_maps(x, gamma):
    global _IDENT
    if _IDENT is None:
        _IDENT = np.eye(P, dtype=np.float16)
    x = np.ascontiguousarray(np.asarray(x, dtype=np.float32)).reshape(B, C, HW)
    gamma = np.asarray(gamma, dtype=np.float32).reshape(1)
    return [
        {"x": x[i * NB:(i + 1) * NB], "gamma": gamma, "ident": _IDENT}
        for i in range(NCORES)
    ]


def run(x, gamma, trace=False, **trace_kwargs):
    from concourse.bass_utils import run_bass_kernel_spmd

    nc = get_bass()
    res = run_bass_kernel_spmd(
        nc, make_in_maps(x, gamma), core_ids=list(range(NCORES)),
        trace=trace, **trace_kwargs,
    )
    out = np.concatenate([res.results[i]["out"] for i in range(NCORES)], axis=0)
    return out.reshape(B, C, H, W), res


def kernel(x, gamma):
    out, _ = run(x, gamma, trace=False)
    return out
